# revision 40
# baseline (speedup 1.0000x reference)
"""Trainium2 Bass kernel for nn_Attention_41566693491235.

Computes, for full inputs (B=256, L=196, R=1024, A=512, D=2048):
    att_h  = h @ W_h + b_h                                  [B, A]
    dot    = einsum("bla,a->bl", tanh(f2 + att_h[:,None,:]), w_a) + b_a
    weight = softmax(dot, axis=1) * mask;  weight /= weight.sum(1, keepdims=True)
    att    = einsum("bl,bld->bd", weight, f1)               [B, D]

Sharding: data-parallel over batch, 32 per core x 8 cores. Weights replicated.
Note b_a and the softmax normalizer cancel exactly in the masked renorm:
    weight = exp(dot - max) * mask / sum(exp(dot - max) * mask)

Implementation notes:
  - Matmuls run in float32r (full-rate PE at moving-dim >= 256, ~2e-4 rel err).
  - The weighted sum packs 4 l-rows x all 32 batches per 128-row contraction
    (rows k = b*4 + j) with block-diagonal stationary operands, so all 32
    output rows accumulate partition-aligned in one PSUM tile and L = 49*4
    divides evenly (no ragged chunks).
  - f1 streams on the sync HWDGE ring into a deep SBUF pool from t=0; f2
    alternates between the scalar HWDGE ring and gpsimd SWDGE; softmax runs
    per 16-batch half as soon as that half's dots land, so only the last
    half's weight redistribution sits on the critical path.
"""

import numpy as np

import concourse.bass as bass
import concourse.bacc as bacc
import concourse.tile as tile
import concourse.mybir as mybir
from concourse import bass_utils

F32 = mybir.dt.float32
F32R = mybir.dt.float32r
AF = mybir.ActivationFunctionType

# Problem shape (hardcoded; kernel.py must be self-contained).
B, L, R, A, D = 256, 196, 1024, 512, 2048
NCORES = 8
BL = B // NCORES          # 32 batches per core
NLC = L // 4              # 49 l-chunks of 4 rows x 32 batches = 128 K-rows
NAC = A // 128            # 4 chunks of the attention-hidden dim
NKC = R // 128            # 8 chunks of the h-feature dim
NDC = D // 512            # 4 free-dim chunks for the output matmuls


def _build_program(f1_bufs: int = 13, f2_bufs: int = 4):
    nc = bacc.Bacc(
        "TRN2",
        target_bir_lowering=False,
        debug=False,
        enable_asserts=False,
        num_devices=NCORES,
    )

    hT = nc.dram_tensor("hT", [R, BL], F32, kind="ExternalInput").ap()
    wh = nc.dram_tensor("wh", [R, A], F32, kind="ExternalInput").ap()
    bh4 = nc.dram_tensor("bh4", [128, NAC], F32, kind="ExternalInput").ap()
    wa4 = nc.dram_tensor("wa4", [128, NAC], F32, kind="ExternalInput").ap()
    # f2h[bp, p, i, ac, l] = f2[2*bp+i, l, ac*128+p]: one contiguous
    # 800KB block per batch-pair, 6.3KB per partition.
    f2h = nc.dram_tensor("f2h", [BL // 2, 128, 2, NAC, L], F32,
                         kind="ExternalInput").ap()
    # f1h[lch, b, j, d] = f1[b, 4*lch+j, d]: each lch tile is a contiguous
    # 1MB block in exactly the SBUF layout (rows k = b*4 + j).
    f1h = nc.dram_tensor("f1h", [NLC, 128, D], F32, kind="ExternalInput").ap()
    # msk2[bs, half, l] = att_masks[16*half + bs, l]
    msk = nc.dram_tensor("msk", [16, 2, L], F32, kind="ExternalInput").ap()
    # bdm[b*4+j, b'] = 1 iff b' == b: the block-diagonal mask.
    bdm = nc.dram_tensor("bdm", [128, BL], F32, kind="ExternalInput").ap()
    att = nc.dram_tensor("att", [BL, D], F32, kind="ExternalOutput").ap()

    with tile.TileContext(nc) as tc:
        with (
            tc.tile_pool(name="const", bufs=1) as cpool,
            tc.tile_pool(name="f2p", bufs=f2_bufs) as f2pool,
            tc.tile_pool(name="ep", bufs=3) as epool,
            tc.tile_pool(name="f1p", bufs=f1_bufs) as f1pool,
            tc.tile_pool(name="small", bufs=1) as spool,
            tc.tile_pool(name="ps", bufs=1, space=bass.MemorySpace.PSUM) as pspool,
            tc.tile_pool(name="psdot", bufs=3, space=bass.MemorySpace.PSUM) as psdot,
            tc.tile_pool(name="dram", bufs=1, space=bass.MemorySpace.DRAM) as dpool,
        ):
            # ---- constants -------------------------------------------------
            hT_t = cpool.tile([128, NKC, BL], F32R)
            nc.sync.dma_start(hT_t[:], hT.bitcast(F32R).rearrange("(kc p) b -> p kc b", p=128))
            wa_t = cpool.tile([128, NAC], F32R)
            nc.sync.dma_start(wa_t[:], wa4.bitcast(F32R)[:])
            bh_t = cpool.tile([128, NAC], F32)
            nc.scalar.dma_start(bh_t[:], bh4[:])
            # W_h rides in f1-pool slots (released after phase 1, so the
            # space is recycled for f1 buffering); split across both HWDGE
            # rings so att_h is ready as early as possible.
            wh_tiles = []
            for kc in range(NKC):
                wht = f1pool.tile([128, A], F32R, tag="f1t", name=f"wh{kc}")
                eng = nc.sync if kc % 2 == 0 else nc.scalar
                eng.dma_start(
                    wht[:], wh.bitcast(F32R)[kc * 128:(kc + 1) * 128, :]
                )
                wh_tiles.append(wht)
            msk_t = cpool.tile([16, 2, L], F32)
            nc.scalar.dma_start(msk_t[:], msk[:])
            bdm_t = cpool.tile([128, BL], F32)
            nc.scalar.dma_start(bdm_t[:], bdm[:])

            # DRAM scratch for the partition-redistribution round-trips.
            dot_dram = dpool.tile([BL, L], F32)
            w_dram = dpool.tile([NLC, BL, 4], F32)

            # ---- phase 1: att_h.T = W_h.T @ h.T (+ b_h) --------------------
            # atth[:, ac, b] holds att_h[b, ac*128 + p] on partition p.
            ps_atth = pspool.tile([128, NAC, BL], F32)
            for mc in range(NAC):
                for kc in range(NKC):
                    nc.tensor.matmul(
                        ps_atth[:, mc, :],
                        wh_tiles[kc][:, mc * 128:(mc + 1) * 128],
                        hT_t[:, kc, :],
                        start=(kc == 0),
                        stop=(kc == NKC - 1),
                    )
            atth = cpool.tile([128, NAC, BL], F32)
            for mc in range(NAC):
                nc.vector.tensor_scalar_add(
                    atth[:, mc, :], ps_atth[:, mc, :], bh_t[:, mc:mc + 1]
                )

            # ---- phase 2: tanh + dot, two batches per matmul ---------------
            # dot[b, l] = sum_a tanh(f2[b,l,a] + att_h[b,a]) * w_a[a]
            def emit_pair(bp):
                f2b = f2pool.tile([128, 2, NAC, L], F32, tag="f2b",
                                  name=f"f2b{bp}")
                nc.scalar.dma_start(f2b[:], f2h[bp])
                # Bias-add on DVE with a free-dim-broadcast AP, then ONE big
                # tanh on ACT (8x fewer ACT ops; the per-op errata overhead
                # dominated the small ones).
                f2s = epool.tile([128, NAC, 2, L], F32, tag="f2s",
                                 name=f"f2s{bp}")
                nc.vector.tensor_add(
                    f2s[:],
                    f2b.rearrange("p i ac l -> p ac i l"),
                    atth[:, :, 2 * bp:2 * bp + 2].to_broadcast(
                        (128, NAC, 2, L)
                    ),
                )
                e2 = epool.tile([128, NAC, 2, L], F32R, tag="e2",
                                name=f"e2{bp}")
                nc.scalar.activation(e2[:], f2s[:], AF.Tanh)
                pd = psdot.tile([1, 2, L], F32, tag="pd", name=f"pd{bp}")
                for ac in range(NAC):
                    nc.tensor.matmul(
                        pd[:], wa_t[:, ac:ac + 1], e2[:, ac, :, :],
                        start=(ac == 0), stop=(ac == NAC - 1),
                    )
                dotflat = spool.tile([1, 2 * L], F32, tag="dotflat", bufs=3,
                                     name=f"dotflat{bp}")
                nc.vector.tensor_copy(dotflat[:], pd[:])
                nc.gpsimd.dma_start(dot_dram[2 * bp:2 * bp + 2, :], dotflat[:])

            # Block-diagonal stationary operand, built per half as soon as
            # that half's weights land. Each half's build covers all 32
            # columns: the mask is zero in the off-half columns, so the
            # two builds tile the full 128 partitions with no memset.
            ldt = spool.tile([128, NLC, BL], F32R)
            w2 = spool.tile([128, NLC], F32)

            # ---- phase 3/4 per 16-batch half: masked softmax + weight
            # redistribution write. Only the last half's chain gates step5.
            def emit_half(hf):
                b0 = 16 * hf
                dott = spool.tile([16, L], F32, tag=f"dott{hf}",
                                  name=f"dott{hf}")
                nc.scalar.dma_start(dott[:], dot_dram[b0:b0 + 16, :])
                negmax = spool.tile([16, 1], F32, tag=f"negmax{hf}",
                                    name=f"negmax{hf}")
                nc.vector.tensor_reduce(
                    negmax[:], dott[:], axis=mybir.AxisListType.X,
                    op=mybir.AluOpType.max, negate=True,
                )
                wexp = spool.tile([16, L], F32, tag=f"wexp{hf}",
                                  name=f"wexp{hf}")
                nc.scalar.activation(wexp[:], dott[:], AF.Exp, bias=negmax[:])
                wm = spool.tile([16, L], F32, tag=f"wm{hf}", name=f"wm{hf}")
                nc.vector.tensor_mul(wm[:], wexp[:], msk_t[:, hf, :])
                ssum = spool.tile([16, 1], F32, tag=f"ssum{hf}",
                                  name=f"ssum{hf}")
                nc.vector.reduce_sum(ssum[:], wm[:], axis=mybir.AxisListType.X)
                sinv = spool.tile([16, 1], F32, tag=f"sinv{hf}",
                                  name=f"sinv{hf}")
                nc.vector.reciprocal(sinv[:], ssum[:])
                wn = spool.tile([16, L], F32, tag=f"wn{hf}", name=f"wn{hf}")
                nc.vector.tensor_scalar_mul(wn[:], wm[:], sinv[:])
                # W2[b*4+j, lch] = wn[b, 4*lch+j], staged through DRAM.
                nc.scalar.dma_start(
                    w_dram[:, b0:b0 + 16, :].rearrange("lch b j -> b lch j"),
                    wn[:],
                )
                p0 = 64 * hf
                nc.scalar.dma_start(
                    w2[p0:p0 + 64, :],
                    w_dram[:, b0:b0 + 16, :].rearrange("lch b j -> (b j) lch"),
                )
                for lch in range(NLC):
                    nc.vector.tensor_scalar_mul(
                        ldt[p0:p0 + 64, lch, :],
                        bdm_t[p0:p0 + 64, :],
                        w2[p0:p0 + 64, lch:lch + 1],
                    )

            for bp in range(8):
                emit_pair(bp)
            emit_half(0)
            for bp in range(8, 16):
                emit_pair(bp)
            emit_half(1)

            # ---- phase 5: att = weight @ f1, all 32 batches per matmul -----
            ps_att = pspool.tile([BL, NDC, 512], F32)
            for lch in range(NLC):
                f1t = f1pool.tile([128, D], F32R, tag="f1t", name=f"f1t{lch}")
                nc.sync.dma_start(f1t[:], f1h.bitcast(F32R)[lch])
                for dc in range(NDC):
                    nc.tensor.matmul(
                        ps_att[:, dc, :],
                        ldt[:, lch, :],
                        f1t[:, dc * 512:(dc + 1) * 512],
                        start=(lch == 0),
                        stop=(lch == NLC - 1),
                    )

            for dc in range(NDC):
                att_sb = spool.tile([BL, 512], F32, tag="att_sb", bufs=2,
                                    name=f"att_sb{dc}")
                nc.vector.tensor_copy(att_sb[:], ps_att[:, dc, :])
                nc.sync.dma_start(att[:, dc * 512:(dc + 1) * 512], att_sb[:])

    nc.compile()
    return nc


_PROGRAM_CACHE = {}


def _get_program():
    if "nc" not in _PROGRAM_CACHE:
        _PROGRAM_CACHE["nc"] = _build_program()
    return _PROGRAM_CACHE["nc"]


def make_in_maps(h, att_feats1, att_feats2, att_masks, W_h, b_h, w_a, b_a):
    h = np.asarray(h, dtype=np.float32)
    att_feats1 = np.asarray(att_feats1, dtype=np.float32)
    att_feats2 = np.asarray(att_feats2, dtype=np.float32)
    att_masks = np.asarray(att_masks, dtype=np.float32)
    W_h = np.ascontiguousarray(np.asarray(W_h, dtype=np.float32))
    b_h = np.asarray(b_h, dtype=np.float32)
    w_a = np.asarray(w_a, dtype=np.float32)
    del b_a  # cancels exactly in the softmax + masked renormalization

    wa4 = np.ascontiguousarray(w_a.reshape(NAC, 128).T)
    bh4 = np.ascontiguousarray(b_h.reshape(NAC, 128).T)
    bdm = np.zeros((128, BL), dtype=np.float32)
    for b in range(BL):
        bdm[b * 4:(b + 1) * 4, b] = 1.0

    in_maps = []
    for c in range(NCORES):
        sl = slice(c * BL, (c + 1) * BL)
        f1c = att_feats1[sl]
        f2c = att_feats2[sl]
        # f2h[bp, p, i, ac, l] = f2[2bp+i, l, ac*128+p]
        f2h = np.ascontiguousarray(
            f2c.reshape(BL // 2, 2, L, NAC, 128).transpose(0, 4, 1, 3, 2)
        )
        # f1h[lch, b, j, d] = f1[b, 4lch+j, d]
        f1h = np.ascontiguousarray(
            f1c.reshape(BL, NLC, 4, D).transpose(1, 0, 2, 3)
        ).reshape(NLC, 128, D)
        in_maps.append({
            "hT": np.ascontiguousarray(h[sl].T),
            "wh": W_h,
            "bh4": bh4,
            "wa4": wa4,
            "f2h": f2h,
            "f1h": f1h,
            "msk": np.ascontiguousarray(
                att_masks[sl].reshape(2, 16, L).transpose(1, 0, 2)
            ),
            "bdm": bdm,
        })
    return in_maps


def kernel(h, att_feats1, att_feats2, att_masks, W_h, b_h, w_a, b_a,
           _trace=False, _return_results=False):
    nc = _get_program()
    in_maps = make_in_maps(h, att_feats1, att_feats2, att_masks, W_h, b_h,
                           w_a, b_a)
    res = bass_utils.run_bass_kernel_spmd(
        nc, in_maps, core_ids=list(range(NCORES)), trace=_trace
    )
    out = np.concatenate([res.results[c]["att"] for c in range(NCORES)], axis=0)
    if _return_results:
        return out, res
    return out


# revision 41
# speedup vs baseline: 1.1900x; 1.1900x over previous
"""Trainium2 Bass kernel for nn_Attention_41566693491235.

Computes, for full inputs (B=256, L=196, R=1024, A=512, D=2048):
    att_h  = h @ W_h + b_h                                  [B, A]
    dot    = einsum("bla,a->bl", tanh(f2 + att_h[:,None,:]), w_a) + b_a
    weight = softmax(dot, axis=1) * mask;  weight /= weight.sum(1, keepdims=True)
    att    = einsum("bl,bld->bd", weight, f1)               [B, D]

Sharding: data-parallel over batch, 32 per core x 8 cores. Weights replicated.
Note b_a and the softmax normalizer cancel exactly in the masked renorm:
    weight = exp(dot - max) * mask / sum(exp(dot - max) * mask)

Implementation notes:
  - Matmuls run in float32r (full-rate PE at moving-dim >= 256, ~2e-4 rel err).
  - The weighted sum packs 4 l-rows x all 32 batches per 128-row contraction
    (rows k = b*4 + j) with block-diagonal stationary operands, so all 32
    output rows accumulate partition-aligned in one PSUM tile and L = 49*4
    divides evenly (no ragged chunks).
  - f1 streams on the sync HWDGE ring into a deep SBUF pool from t=0; f2
    alternates between the scalar HWDGE ring and gpsimd SWDGE; softmax runs
    per 16-batch half as soon as that half's dots land, so only the last
    half's weight redistribution sits on the critical path.
"""

import numpy as np

import concourse.bass as bass
import concourse.bacc as bacc
import concourse.tile as tile
import concourse.mybir as mybir
from concourse import bass_utils

F32 = mybir.dt.float32
F32R = mybir.dt.float32r
AF = mybir.ActivationFunctionType

# Problem shape (hardcoded; kernel.py must be self-contained).
B, L, R, A, D = 256, 196, 1024, 512, 2048
NCORES = 8
BL = B // NCORES          # 32 batches per core
NLC = L // 4              # 49 l-chunks of 4 rows x 32 batches = 128 K-rows
NAC = A // 128            # 4 chunks of the attention-hidden dim
NKC = R // 128            # 8 chunks of the h-feature dim
NDC = D // 512            # 4 free-dim chunks for the output matmuls


def _build_program(f1_bufs: int = 13, f2_bufs: int = 4):
    nc = bacc.Bacc(
        "TRN2",
        target_bir_lowering=False,
        debug=False,
        enable_asserts=False,
        num_devices=NCORES,
    )

    hT = nc.dram_tensor("hT", [R, BL], F32, kind="ExternalInput").ap()
    wh = nc.dram_tensor("wh", [R, A], F32, kind="ExternalInput").ap()
    bh4 = nc.dram_tensor("bh4", [128, NAC], F32, kind="ExternalInput").ap()
    wa4 = nc.dram_tensor("wa4", [128, NAC], F32, kind="ExternalInput").ap()
    # f2h[bp, p, i, ac, l] = f2[2*bp+i, l, ac*128+p]: one contiguous
    # 800KB block per batch-pair, 6.3KB per partition.
    f2h = nc.dram_tensor("f2h", [BL // 2, 128, 2, NAC, L], F32,
                         kind="ExternalInput").ap()
    # f1h[lch, b, j, d] = f1[b, 4*lch+j, d]: each lch tile is a contiguous
    # 1MB block in exactly the SBUF layout (rows k = b*4 + j).
    f1h = nc.dram_tensor("f1h", [NLC, 128, D], F32, kind="ExternalInput").ap()
    # msk2[bs, half, l] = att_masks[16*half + bs, l]
    msk = nc.dram_tensor("msk", [16, 2, L], F32, kind="ExternalInput").ap()
    # bdm[b*4+j, b'] = 1 iff b' == b: the block-diagonal mask.
    bdm = nc.dram_tensor("bdm", [128, BL], F32, kind="ExternalInput").ap()
    att = nc.dram_tensor("att", [BL, D], F32, kind="ExternalOutput").ap()

    with tile.TileContext(nc) as tc:
        with (
            tc.tile_pool(name="const", bufs=1) as cpool,
            tc.tile_pool(name="f2p", bufs=f2_bufs) as f2pool,
            tc.tile_pool(name="ep", bufs=3) as epool,
            tc.tile_pool(name="f1p", bufs=f1_bufs) as f1pool,
            tc.tile_pool(name="small", bufs=1) as spool,
            tc.tile_pool(name="ps", bufs=1, space=bass.MemorySpace.PSUM) as pspool,
            tc.tile_pool(name="psdot", bufs=3, space=bass.MemorySpace.PSUM) as psdot,
            tc.tile_pool(name="dram", bufs=1, space=bass.MemorySpace.DRAM) as dpool,
        ):
            # ---- constants -------------------------------------------------
            hT_t = cpool.tile([128, NKC, BL], F32R)
            nc.sync.dma_start(hT_t[:], hT.bitcast(F32R).rearrange("(kc p) b -> p kc b", p=128))
            wa_t = cpool.tile([128, NAC], F32R)
            nc.sync.dma_start(wa_t[:], wa4.bitcast(F32R)[:])
            bh_t = cpool.tile([128, NAC], F32)
            nc.scalar.dma_start(bh_t[:], bh4[:])
            # W_h rides in f1-pool slots (released after phase 1, so the
            # space is recycled for f1 buffering); split across both HWDGE
            # rings so att_h is ready as early as possible.
            wh_tiles = []
            for kc in range(NKC):
                wht = f1pool.tile([128, A], F32R, tag="f1t", name=f"wh{kc}")
                eng = nc.sync if kc % 2 == 0 else nc.scalar
                eng.dma_start(
                    wht[:], wh.bitcast(F32R)[kc * 128:(kc + 1) * 128, :]
                )
                wh_tiles.append(wht)
            msk_t = cpool.tile([16, 2, L], F32)
            nc.scalar.dma_start(msk_t[:], msk[:])
            bdm_t = cpool.tile([128, BL], F32)
            nc.scalar.dma_start(bdm_t[:], bdm[:])

            # DRAM scratch for the partition-redistribution round-trips.
            dot_dram = dpool.tile([BL, L], F32)
            w_dram = dpool.tile([NLC, BL, 4], F32)

            # ---- phase 1: att_h.T = W_h.T @ h.T (+ b_h) --------------------
            # atth[:, ac, b] holds att_h[b, ac*128 + p] on partition p.
            ps_atth = pspool.tile([128, NAC, BL], F32)
            for mc in range(NAC):
                for kc in range(NKC):
                    nc.tensor.matmul(
                        ps_atth[:, mc, :],
                        wh_tiles[kc][:, mc * 128:(mc + 1) * 128],
                        hT_t[:, kc, :],
                        start=(kc == 0),
                        stop=(kc == NKC - 1),
                    )
            atth = cpool.tile([128, NAC, BL], F32)
            for mc in range(NAC):
                nc.vector.tensor_scalar_add(
                    atth[:, mc, :], ps_atth[:, mc, :], bh_t[:, mc:mc + 1]
                )

            # ---- phase 2: tanh + dot, two batches per matmul ---------------
            # dot[b, l] = sum_a tanh(f2[b,l,a] + att_h[b,a]) * w_a[a]
            # f2 issues ride gpsimd's own stream (4-deep prologue) so they
            # are never serialized behind the tanh/dot consumer chain.
            f2_tiles = {}

            def issue_f2(bp):
                f2b = f2pool.tile([128, 2, NAC, L], F32, tag="f2b",
                                  name=f"f2b{bp}")
                nc.gpsimd.dma_start(f2b[:], f2h[bp])
                f2_tiles[bp] = f2b

            for bp in range(4):
                issue_f2(bp)

            def emit_pair(bp):
                f2b = f2_tiles.pop(bp)
                if bp + 4 < BL // 2:
                    issue_f2(bp + 4)
                # Bias-add on DVE with a free-dim-broadcast AP, then ONE big
                # tanh on ACT (8x fewer ACT ops; the per-op errata overhead
                # dominated the small ones).
                f2s = epool.tile([128, NAC, 2, L], F32, tag="f2s",
                                 name=f"f2s{bp}")
                nc.vector.tensor_add(
                    f2s[:],
                    f2b.rearrange("p i ac l -> p ac i l"),
                    atth[:, :, 2 * bp:2 * bp + 2].to_broadcast(
                        (128, NAC, 2, L)
                    ),
                )
                e2 = epool.tile([128, NAC, 2, L], F32R, tag="e2",
                                name=f"e2{bp}")
                nc.scalar.activation(e2[:], f2s[:], AF.Tanh)
                pd = psdot.tile([1, 2, L], F32, tag="pd", name=f"pd{bp}")
                for ac in range(NAC):
                    nc.tensor.matmul(
                        pd[:], wa_t[:, ac:ac + 1], e2[:, ac, :, :],
                        start=(ac == 0), stop=(ac == NAC - 1),
                    )
                dotflat = spool.tile([1, 2 * L], F32, tag="dotflat", bufs=3,
                                     name=f"dotflat{bp}")
                nc.vector.tensor_copy(dotflat[:], pd[:])
                nc.gpsimd.dma_start(dot_dram[2 * bp:2 * bp + 2, :], dotflat[:])

            # Block-diagonal stationary operand, built per half as soon as
            # that half's weights land. Each half's build covers all 32
            # columns: the mask is zero in the off-half columns, so the
            # two builds tile the full 128 partitions with no memset.
            ldt = spool.tile([128, NLC, BL], F32R)
            w2 = spool.tile([128, NLC], F32)

            # ---- phase 3/4 per 16-batch half: masked softmax + weight
            # redistribution write. Only the last half's chain gates step5.
            def emit_half(hf):
                b0 = 16 * hf
                dott = spool.tile([16, L], F32, tag=f"dott{hf}",
                                  name=f"dott{hf}")
                nc.scalar.dma_start(dott[:], dot_dram[b0:b0 + 16, :])
                negmax = spool.tile([16, 1], F32, tag=f"negmax{hf}",
                                    name=f"negmax{hf}")
                nc.vector.tensor_reduce(
                    negmax[:], dott[:], axis=mybir.AxisListType.X,
                    op=mybir.AluOpType.max, negate=True,
                )
                wexp = spool.tile([16, L], F32, tag=f"wexp{hf}",
                                  name=f"wexp{hf}")
                nc.scalar.activation(wexp[:], dott[:], AF.Exp, bias=negmax[:])
                wm = spool.tile([16, L], F32, tag=f"wm{hf}", name=f"wm{hf}")
                nc.vector.tensor_mul(wm[:], wexp[:], msk_t[:, hf, :])
                ssum = spool.tile([16, 1], F32, tag=f"ssum{hf}",
                                  name=f"ssum{hf}")
                nc.vector.reduce_sum(ssum[:], wm[:], axis=mybir.AxisListType.X)
                sinv = spool.tile([16, 1], F32, tag=f"sinv{hf}",
                                  name=f"sinv{hf}")
                nc.vector.reciprocal(sinv[:], ssum[:])
                wn = spool.tile([16, L], F32, tag=f"wn{hf}", name=f"wn{hf}")
                nc.vector.tensor_scalar_mul(wn[:], wm[:], sinv[:])
                # W2[b*4+j, lch] = wn[b, 4*lch+j], staged through DRAM.
                nc.scalar.dma_start(
                    w_dram[:, b0:b0 + 16, :].rearrange("lch b j -> b lch j"),
                    wn[:],
                )
                p0 = 64 * hf
                nc.scalar.dma_start(
                    w2[p0:p0 + 64, :],
                    w_dram[:, b0:b0 + 16, :].rearrange("lch b j -> (b j) lch"),
                )
                for lch in range(NLC):
                    nc.vector.tensor_scalar_mul(
                        ldt[p0:p0 + 64, lch, :],
                        bdm_t[p0:p0 + 64, :],
                        w2[p0:p0 + 64, lch:lch + 1],
                    )

            for bp in range(8):
                emit_pair(bp)
            emit_half(0)
            for bp in range(8, 16):
                emit_pair(bp)
            emit_half(1)

            # ---- phase 5: att = weight @ f1, all 32 batches per matmul -----
            ps_att = pspool.tile([BL, NDC, 512], F32)
            for lch in range(NLC):
                f1t = f1pool.tile([128, D], F32R, tag="f1t", name=f"f1t{lch}")
                nc.sync.dma_start(f1t[:], f1h.bitcast(F32R)[lch])
                for dc in range(NDC):
                    nc.tensor.matmul(
                        ps_att[:, dc, :],
                        ldt[:, lch, :],
                        f1t[:, dc * 512:(dc + 1) * 512],
                        start=(lch == 0),
                        stop=(lch == NLC - 1),
                    )

            for dc in range(NDC):
                att_sb = spool.tile([BL, 512], F32, tag="att_sb", bufs=2,
                                    name=f"att_sb{dc}")
                nc.vector.tensor_copy(att_sb[:], ps_att[:, dc, :])
                nc.sync.dma_start(att[:, dc * 512:(dc + 1) * 512], att_sb[:])

    nc.compile()
    return nc


_PROGRAM_CACHE = {}


def _get_program():
    if "nc" not in _PROGRAM_CACHE:
        _PROGRAM_CACHE["nc"] = _build_program()
    return _PROGRAM_CACHE["nc"]


def make_in_maps(h, att_feats1, att_feats2, att_masks, W_h, b_h, w_a, b_a):
    h = np.asarray(h, dtype=np.float32)
    att_feats1 = np.asarray(att_feats1, dtype=np.float32)
    att_feats2 = np.asarray(att_feats2, dtype=np.float32)
    att_masks = np.asarray(att_masks, dtype=np.float32)
    W_h = np.ascontiguousarray(np.asarray(W_h, dtype=np.float32))
    b_h = np.asarray(b_h, dtype=np.float32)
    w_a = np.asarray(w_a, dtype=np.float32)
    del b_a  # cancels exactly in the softmax + masked renormalization

    wa4 = np.ascontiguousarray(w_a.reshape(NAC, 128).T)
    bh4 = np.ascontiguousarray(b_h.reshape(NAC, 128).T)
    bdm = np.zeros((128, BL), dtype=np.float32)
    for b in range(BL):
        bdm[b * 4:(b + 1) * 4, b] = 1.0

    in_maps = []
    for c in range(NCORES):
        sl = slice(c * BL, (c + 1) * BL)
        f1c = att_feats1[sl]
        f2c = att_feats2[sl]
        # f2h[bp, p, i, ac, l] = f2[2bp+i, l, ac*128+p]
        f2h = np.ascontiguousarray(
            f2c.reshape(BL // 2, 2, L, NAC, 128).transpose(0, 4, 1, 3, 2)
        )
        # f1h[lch, b, j, d] = f1[b, 4lch+j, d]
        f1h = np.ascontiguousarray(
            f1c.reshape(BL, NLC, 4, D).transpose(1, 0, 2, 3)
        ).reshape(NLC, 128, D)
        in_maps.append({
            "hT": np.ascontiguousarray(h[sl].T),
            "wh": W_h,
            "bh4": bh4,
            "wa4": wa4,
            "f2h": f2h,
            "f1h": f1h,
            "msk": np.ascontiguousarray(
                att_masks[sl].reshape(2, 16, L).transpose(1, 0, 2)
            ),
            "bdm": bdm,
        })
    return in_maps


def kernel(h, att_feats1, att_feats2, att_masks, W_h, b_h, w_a, b_a,
           _trace=False, _return_results=False):
    nc = _get_program()
    in_maps = make_in_maps(h, att_feats1, att_feats2, att_masks, W_h, b_h,
                           w_a, b_a)
    res = bass_utils.run_bass_kernel_spmd(
        nc, in_maps, core_ids=list(range(NCORES)), trace=_trace
    )
    out = np.concatenate([res.results[c]["att"] for c in range(NCORES)], axis=0)
    if _return_results:
        return out, res
    return out
